# revision 2
# baseline (speedup 1.0000x reference)
"""Trainium2 Bass kernel for nn_CopresheafLayer (GNN message passing).

Math (reference):
    h     = silu(rbf @ f1_w.T + f1_b)                    # [E, 128]
    phi   = (h @ f2_w.T + f2_b) * envelope[:, None]      # [E, 64]
    msg   = (phi * (x[src] @ W_send.T)) @ W_recv         # [E, 128]
    agg   = segment_sum(msg, tgt, N)                     # [N, 128]
    gate  = silu(agg @ g1_w.T + g1_b) @ g2_w.T + g2_b
    y     = LayerNorm(x + gate) * ln_g + ln_b

Key algebra used by the kernel:
  * (sum_e stalk_msg_e) @ W_recv  ==  (sum_e stalk_msg_e) @ W_recv: the
    [E,64]@[64,128] matmul is hoisted to per-node; edges only touch the
    64-dim stalk space.
  * x[src] @ W_send.T == (x @ W_send.T)[src]: project once to xs=[N,64] f32,
    gather 256B rows per edge with dma_gather.
  * envelope is folded into the one-hot scatter matrix S (S = onehot * env).
  * f2_b: scatter BOTH phi*xs and raw xs (one [128,128] stationary operand),
    fold f2_b into an augmented W_recv in the per-node tail:
        agg @ W_recv = [scat_msg ; scat_xs] @ [W_recv ; f2_b[:,None]*W_recv]

Sharding: edges bucketed by target-node range (N/8 nodes per core), sorted by
tgt -> each core owns its output slice; no collectives.  segment_sum is a
sequence of one-hot matmuls accumulating into per-128-node-window PSUM
columns.  Edges are grouped (window, src-quarter)-pure and 128-padded so each
(4-window-block, quarter) is one contiguous dma_gather from an int16-indexed
shard of the xs table.
"""

import math
import os
import sys

import numpy as np

sys.path.insert(0, "/opt/trn_rl_repo")

P = 128  # SBUF partitions
D = 128  # d_node
DS = 64  # d_stalk
R = 32  # n_rbf
TB = 8  # tiles per phi batch (1024 edges)
WB = 4  # windows per scatter block
NQ = 4  # xs-table shards (int16 index range)


# --------------------------------------------------------------------------
# Host-side preprocessing (index manipulation only -- no model FLOPs)
# --------------------------------------------------------------------------

def _preprocess(x, edge_index, rbf, envelope, W_send, W_recv, f1_w, f1_b,
                f2_w, f2_b, g1_w, g1_b, g2_w, g2_b, ln_g, ln_b, n_cores):
    import ml_dtypes

    bf16 = ml_dtypes.bfloat16
    N = x.shape[0]
    E = edge_index.shape[1]
    assert N % n_cores == 0
    NS = N // n_cores                 # nodes per core
    NW = (NS + P - 1) // P            # 128-node windows per core
    NCHUNK = -(-N // P)
    # multiple of 4*32 so each xs-table quarter is whole 32-chunk batches
    NCHUNK = -(-NCHUNK // 128) * 128
    SHARD = (P * NCHUNK) // NQ        # xs-table rows per int16 shard
    assert SHARD <= 32768

    src = np.asarray(edge_index[0]).astype(np.int64)
    tgt = np.asarray(edge_index[1]).astype(np.int64)
    env = np.asarray(envelope, dtype=np.float32)
    rbf = np.asarray(rbf, dtype=np.float32)

    order = np.argsort(tgt, kind="stable")
    tgt_s, src_s, env_s = tgt[order], src[order], env[order]
    q_s = src_s // SHARD                          # shard per edge (node order)
    core_of = tgt_s // NS
    win_s = (tgt_s % NS) // P

    # per (core, window, quarter) counts -> shared group structure
    cnt = np.zeros((n_cores, NW, NQ), dtype=np.int64)
    np.add.at(cnt, (core_of, win_s, q_s), 1)
    G = -(-cnt.max(axis=0) // P)                  # tiles per (window, quarter)

    # block/quarter-major tile layout
    nblk = -(-NW // WB)
    grp_start = np.zeros((NW, NQ), dtype=np.int64)
    runs = []          # (q, tile_start, ntiles) per (block, quarter)
    win_first = np.full(NW, -1, dtype=np.int64)
    win_last = np.zeros(NW, dtype=np.int64)
    tile_win = []
    t = 0
    for b in range(nblk):
        ws = range(b * WB, min((b + 1) * WB, NW))
        for q in range(NQ):
            r0 = t
            for w in ws:
                g = int(G[w, q])
                if g == 0:
                    continue
                grp_start[w, q] = t
                if win_first[w] < 0:
                    win_first[w] = t
                win_last[w] = t + g - 1
                tile_win += [w] * g
                t += g
            if b == nblk - 1 and q == NQ - 1:
                pad = (-t) % TB                   # T multiple of TB
                if pad:
                    w = NW - 1
                    tile_win += [w] * pad
                    win_last[w] = t + pad - 1
                    t += pad
            if t > r0:
                runs.append((q, r0, t - r0))
    T = t
    assert all(f >= 0 for f in win_first)

    # msg-op segments: phi batches split at run boundaries
    run_of_tile = np.zeros(T, dtype=np.int64)
    run_t0 = np.zeros(len(runs), dtype=np.int64)
    for ri, (q, r0, n) in enumerate(runs):
        run_of_tile[r0:r0 + n] = ri
        run_t0[ri] = r0
    segments = []      # per batch: list of (j_lo, j_hi, run_idx)
    for bph in range(T // TB):
        segs, t0 = [], bph * TB
        j = 0
        while j < TB:
            ri = run_of_tile[t0 + j]
            j2 = j
            while j2 < TB and run_of_tile[t0 + j2] == ri:
                j2 += 1
            segs.append((j, j2, int(ri)))
            j = j2
        segments.append(segs)

    per_core = []
    for c in range(n_cores):
        slots = T * P
        src_slot = np.zeros(slots, dtype=np.int64)
        env_slot = np.zeros(slots, dtype=np.float32)
        tgtl_slot = np.zeros(slots, dtype=np.float32)
        sel = core_of == c
        tw, tq = win_s[sel], q_s[sel]
        # position within the (w, q) group (vectorized cumcount)
        gkey = tw * NQ + tq
        pos = np.zeros(len(gkey), dtype=np.int64)
        so = np.argsort(gkey, kind="stable")
        gs = gkey[so]
        pos[so] = np.arange(len(gs)) - np.searchsorted(gs, gs)
        slot = grp_start[tw, tq] * P + pos
        src_slot[slot] = src_s[sel]
        env_slot[slot] = env_s[sel]
        tgtl_slot[slot] = (tgt_s[sel] - c * NS - tw * P).astype(np.float32)
        rbf_slot = np.zeros((slots, R), dtype=np.float32)
        rbf_slot[slot] = rbf[order[sel]]

        rel = (src_slot % SHARD).astype(np.int16)
        # wrapped idx layout: idx i -> partition i%16, col i//16, replicated
        idxw = rel.reshape(T * TB, 16).T            # [16, T*8]
        idxw = np.tile(idxw, (8, 1))                # [128, T*8]
        maps = {
            "gidxw": np.ascontiguousarray(idxw),
            "tgtlenv": np.ascontiguousarray(np.concatenate(
                [tgtl_slot.reshape(T, P).T, env_slot.reshape(T, P).T],
                axis=1).astype(bf16)),
            # packed rbf^T: [32a+k, g*128+j] = rbf_slot[g*512+a*128+j, k]
            "rbfp": np.ascontiguousarray(
                rbf_slot.reshape(T // 4, 4, P, R).transpose(1, 3, 0, 2)
                .reshape(P, (T // 4) * P).astype(bf16)),
            "xres": np.ascontiguousarray(np.concatenate([
                np.asarray(x[c * NS:(c + 1) * NS], np.float32)
                + np.asarray(g2_b, np.float32)[None, :],
                np.zeros((NW * P - NS, x.shape[1]), np.float32)], axis=0)),
        }
        per_core.append(maps)

    xT = np.zeros((D, NCHUNK * P), dtype=bf16)
    xT[:, :N] = np.asarray(x, np.float32).T.astype(bf16)
    f2b = np.asarray(f2_b, np.float32)
    wrecv = np.asarray(W_recv, np.float32)
    f1wT = np.asarray(f1_w, np.float32).T           # [32, 128]
    f1wTp = np.zeros((P, 4 * P), dtype=np.float32)  # zero-padded copies
    for a in range(4):
        f1wTp[a * R:(a + 1) * R, a * P:(a + 1) * P] = f1wT
    shared = {
        "xT": xT,
        "f1wTp": f1wTp.astype(bf16),
        "f2wT": np.ascontiguousarray(np.asarray(f2_w, np.float32).T.astype(bf16)),
        "wrecvaug": np.ascontiguousarray(
            np.vstack([wrecv, f2b[:, None] * wrecv]).astype(bf16)),
        "g1wT": np.ascontiguousarray(np.asarray(g1_w, np.float32).T.astype(bf16)),
        "g2wT": np.ascontiguousarray(np.asarray(g2_w, np.float32).T.astype(bf16)),
        "wsendT": np.ascontiguousarray(np.asarray(W_send, np.float32).T.astype(bf16)),
        "f1b": np.asarray(f1_b, np.float32).reshape(D, 1).copy(),
        "g1b": np.asarray(g1_b, np.float32).reshape(D, 1).copy(),
        "iota": np.ascontiguousarray(
            np.broadcast_to(np.arange(P, dtype=np.float32)[None, :], (P, P))
            .astype(bf16)),
        "iota8": np.ascontiguousarray(
            np.broadcast_to(
                np.arange(P, dtype=np.float32)[None, None, :],
                (P, TB, P)).reshape(P, TB * P).astype(bf16)),
    }
    ln_g = np.asarray(ln_g, np.float32)
    ln_b = np.asarray(ln_b, np.float32)
    ln_trivial = bool(np.all(ln_g == 1.0) and np.all(ln_b == 0.0))
    if not ln_trivial:
        shared["lngrow"] = ln_g.reshape(1, D).copy()
        shared["lnbrow"] = ln_b.reshape(1, D).copy()

    cfg = dict(N=N, E=E, NS=NS, NW=NW, NCHUNK=NCHUNK, T=T, SHARD=SHARD,
               runs=runs, segments=segments,
               win_first=win_first.tolist(), win_last=win_last.tolist(),
               tile_win=[int(v) for v in tile_win], ln_trivial=ln_trivial,
               n_cores=n_cores)
    return cfg, shared, per_core


# --------------------------------------------------------------------------
# Bass program (identical across cores)
# --------------------------------------------------------------------------

def _build_program(cfg):
    from contextlib import ExitStack

    import concourse.bacc as bacc
    import concourse.bass as bass
    import concourse.tile as tile
    from concourse import mybir

    f32 = mybir.dt.float32
    bf = mybir.dt.bfloat16
    i16 = mybir.dt.int16
    Alu = mybir.AluOpType
    Act = mybir.ActivationFunctionType

    NS, NW, NCH, T = cfg["NS"], cfg["NW"], cfg["NCHUNK"], cfg["T"]
    SHARD = cfg["SHARD"]
    runs = cfg["runs"]
    segments = cfg["segments"]
    win_first, win_last = cfg["win_first"], cfg["win_last"]
    tile_win = cfg["tile_win"]
    last_nw = NS - P * (NW - 1)

    nc = bacc.Bacc("TRN2", target_bir_lowering=False, debug=False,
                   num_swdge_queues=4)

    rbfp = nc.dram_tensor("rbfp", [P, (T // 4) * P], bf, kind="ExternalInput")
    gidxw_d = nc.dram_tensor("gidxw", [P, T * TB], i16, kind="ExternalInput")
    tgtlenv_d = nc.dram_tensor("tgtlenv", [P, 2 * T], bf, kind="ExternalInput")
    iota8_d = nc.dram_tensor("iota8", [P, TB * P], bf, kind="ExternalInput")
    xT_d = nc.dram_tensor("xT", [D, NCH * P], bf, kind="ExternalInput")
    xres_d = nc.dram_tensor("xres", [NW * P, D], f32, kind="ExternalInput")
    f1wTp_d = nc.dram_tensor("f1wTp", [P, 4 * P], bf, kind="ExternalInput")
    f2wT_d = nc.dram_tensor("f2wT", [D, DS], bf, kind="ExternalInput")
    wrecvaug_d = nc.dram_tensor("wrecvaug", [P, D], bf, kind="ExternalInput")
    g1wT_d = nc.dram_tensor("g1wT", [D, D], bf, kind="ExternalInput")
    g2wT_d = nc.dram_tensor("g2wT", [D, D], bf, kind="ExternalInput")
    wsendT_d = nc.dram_tensor("wsendT", [D, DS], bf, kind="ExternalInput")
    f1b_d = nc.dram_tensor("f1b", [D, 1], f32, kind="ExternalInput")
    g1b_d = nc.dram_tensor("g1b", [D, 1], f32, kind="ExternalInput")
    iota_d = nc.dram_tensor("iota", [P, P], bf, kind="ExternalInput")
    if not cfg["ln_trivial"]:
        lngrow_d = nc.dram_tensor("lngrow", [1, D], f32, kind="ExternalInput")
        lnbrow_d = nc.dram_tensor("lnbrow", [1, D], f32, kind="ExternalInput")
    y_d = nc.dram_tensor("y", [NW * P, D], f32, kind="ExternalOutput")

    with ExitStack() as ctx:
        tc = ctx.enter_context(tile.TileContext(nc))
        consts = ctx.enter_context(tc.tile_pool(name="consts", bufs=1))
        dram = ctx.enter_context(tc.tile_pool(name="dram", bufs=1, space="DRAM"))

        def load_const(dt_tensor, shape, dtype, name):
            t = consts.tile(shape, dtype, name=name)
            nc.sync.dma_start(out=t, in_=dt_tensor[:, :])
            return t

        f1wTp_sb = load_const(f1wTp_d, [P, 4 * P], bf, "f1wTp_sb")
        f2wT_sb = load_const(f2wT_d, [D, DS], bf, "f2wT_sb")
        wrecvaug_sb = load_const(wrecvaug_d, [P, D], bf, "wrecvaug_sb")
        g1wT_sb = load_const(g1wT_d, [D, D], bf, "g1wT_sb")
        g2wT_sb = load_const(g2wT_d, [D, D], bf, "g2wT_sb")
        wsendT_sb = load_const(wsendT_d, [D, DS], bf, "wsendT_sb")
        f1b_sb = load_const(f1b_d, [D, 1], f32, "f1b_sb")
        g1b_sb = load_const(g1b_d, [D, 1], f32, "g1b_sb")
        iota_sb = load_const(iota_d, [P, P], bf, "iota_sb")
        eps_sb = consts.tile([P, 1], f32, name="eps_sb")
        nc.vector.memset(eps_sb, 1e-5)
        gidxw_sb = load_const(gidxw_d, [P, T * TB], i16, "gidxw_sb")
        tgtlenv_sb = load_const(tgtlenv_d, [P, 2 * T], bf, "tgtlenv_sb")
        tgtl_sb = tgtlenv_sb[:, 0:T]
        envs_sb = tgtlenv_sb[:, T:2 * T]
        iota8_sb = load_const(iota8_d, [P, TB * P], bf, "iota8_sb")
        # tiny DVE touch of iota so the first S-build op doesn't carry its
        # DMA wait (TensorScalar ISA allows only 2 sync waits)
        iota_touch = consts.tile([P, 1], f32, name="iota_touch")
        nc.vector.tensor_copy(out=iota_touch, in_=iota_sb[:, 0:1])
        if not cfg["ln_trivial"]:
            lng_bc = consts.tile([P, D], f32, name="lng_bc")
            nc.sync.dma_start(
                out=lng_bc,
                in_=bass.AP(tensor=lngrow_d, offset=0, ap=[[0, P], [1, D]]))
            lnb_bc = consts.tile([P, D], f32, name="lnb_bc")
            nc.sync.dma_start(
                out=lnb_bc,
                in_=bass.AP(tensor=lnbrow_d, offset=0, ap=[[0, P], [1, D]]))

        # xs table in DRAM (f32), node-order rows, one tile per quarter so
        # gathers for quarter q only depend on 1/4 of phase 1
        xsdq = [dram.tile([SHARD, DS], f32, name=f"xsdq{q}")
                for q in range(NQ)]

        # ---------------- phase 1: xs = x @ W_send.T (f32) ----------------
        # Emitted lazily: quarters 0-1 up front, the rest interleaved into
        # the phase-2 batch loop so engine queues aren't clogged by phase 1.
        PH1TB = 32
        qbat = (NCH // PH1TB) // NQ    # 16-chunk batches per quarter
        ph1 = ctx.enter_context(tc.tile_pool(name="ph1", bufs=3))

        tps = ctx.enter_context(tc.tile_pool(name="tps", bufs=2,
                                              space="PSUM"))

        def emit_ph1_batch(cb):
            q = cb // qbat
            xt_sb = ph1.tile([D, PH1TB * P], bf, tag="xt")
            nc.sync.dma_start(
                out=xt_sb,
                in_=xT_d[:, cb * PH1TB * P:(cb + 1) * PH1TB * P])
            xs_sb = ph1.tile([P, PH1TB * DS], f32, tag="xs")
            for h in range(PH1TB // TB):
                ps = tps.tile([P, WB * P], f32, tag="tp")
                for j in range(TB):
                    nc.tensor.matmul(
                        out=ps[:, j * DS:(j + 1) * DS],
                        lhsT=xt_sb[:, (h * TB + j) * P:
                                   (h * TB + j + 1) * P],
                        rhs=wsendT_sb, start=True, stop=True)
                nc.scalar.activation(
                    out=xs_sb[:, h * TB * DS:(h + 1) * TB * DS],
                    in_=ps, func=Act.Copy)
            # write to node-order rows of quarter q: (p, j, s) ->
            # local row (cb*PH1TB+j)*128 + p - q*SHARD
            nc.sync.dma_start(
                out=bass.AP(tensor=xsdq[q].tensor,
                            offset=(cb - q * qbat) * PH1TB * P * DS,
                            ap=[[DS, P], [P * DS, PH1TB], [1, DS]]),
                in_=xs_sb.rearrange("p (j s) -> p j s", s=DS))

        ph1_iter = iter(range(NCH // PH1TB))
        ph1_done = [0]

        def emit_ph1(n):
            for _ in range(n):
                cb = next(ph1_iter, None)
                if cb is None:
                    return
                emit_ph1_batch(cb)
                ph1_done[0] += 1

        emit_ph1(NCH // PH1TB)         # all of phase 1 up front

        # ---------------- phase 2: edges ----------------
        catp = ctx.enter_context(tc.tile_pool(name="catp", bufs=3))
        xrun = ctx.enter_context(tc.tile_pool(name="xrun", bufs=4))
        rbfsp = ctx.enter_context(tc.tile_pool(name="rbfsp", bufs=4))
        hp = ctx.enter_context(tc.tile_pool(name="hp", bufs=3))
        phip = ctx.enter_context(tc.tile_pool(name="phip", bufs=3))
        sp = ctx.enter_context(tc.tile_pool(name="sp", bufs=4))
        tailp = ctx.enter_context(tc.tile_pool(name="tailp", bufs=2))
        hps = ctx.enter_context(tc.tile_pool(name="hps", bufs=2, space="PSUM"))
        phips = ctx.enter_context(tc.tile_pool(name="phips", bufs=2, space="PSUM"))
        scps = ctx.enter_context(tc.tile_pool(name="scps", bufs=2, space="PSUM"))

        max_run = max(n for (_, _, n) in runs)
        nblk = -(-NW // WB)
        blk_last = [max(win_last[w]
                        for w in range(b * WB, min((b + 1) * WB, NW)))
                    for b in range(nblk)]

        def tail_block(blk, psc_tile):
            w0 = blk * WB
            nwin = min(WB, NW - w0)
            W = nwin * P
            sc_sb = tailp.tile([P, WB * P], bf, tag="sc")
            nc.scalar.activation(out=sc_sb[:, :W], in_=psc_tile[:, :W],
                                 func=Act.Copy)
            pB = tps.tile([P, WB * P], f32, tag="tp")
            nc.tensor.matmul(out=pB[:, :W], lhsT=wrecvaug_sb,
                             rhs=sc_sb[:, :W], start=True, stop=True)
            B_sb = tailp.tile([P, WB * P], bf, tag="B")
            nc.scalar.activation(out=B_sb[:, :W], in_=pB[:, :W], func=Act.Copy)
            pC = tps.tile([P, WB * P], f32, tag="tp")
            nc.tensor.matmul(out=pC[:, :W], lhsT=g1wT_sb, rhs=B_sb[:, :W],
                             start=True, stop=True)
            C_sb = tailp.tile([P, WB * P], bf, tag="C")
            nc.scalar.activation(out=C_sb[:, :W], in_=pC[:, :W], func=Act.Silu,
                                 bias=g1b_sb[:, 0:1], scale=1.0)
            pG = tps.tile([P, WB * P], f32, tag="tp")
            for wi in range(nwin):
                nc.tensor.matmul(out=pG[:, wi * P:(wi + 1) * P],
                                 lhsT=C_sb[:, wi * P:(wi + 1) * P],
                                 rhs=g2wT_sb, start=True, stop=True)
            xw = tailp.tile([P, WB, D], f32, tag="xw")
            nc.sync.dma_start(
                out=xw[:, :nwin, :],
                in_=bass.AP(tensor=xres_d, offset=w0 * P * D,
                            ap=[[D, P], [P * D, nwin], [1, D]]))
            u = tailp.tile([P, WB, D], f32, tag="u")
            nc.vector.scalar_tensor_tensor(
                out=u[:, :nwin, :],
                in0=pG.rearrange("p (a b) -> p a b", b=D)[:, :nwin, :],
                scalar=1.0, in1=xw[:, :nwin, :], op0=Alu.mult, op1=Alu.add)
            st4 = tailp.tile([P, WB, 6], f32, tag="st4")
            mv4 = tailp.tile([P, WB, 2], f32, tag="mv4")
            for wi in range(nwin):
                nc.vector.bn_stats(out=st4[:, wi, :], in_=u[:, wi, :])
                nc.vector.bn_aggr(out=mv4[:, wi, :], in_=st4[:, wi, :])
            sd4 = tailp.tile([P, WB], f32, tag="sd4")
            nc.scalar.activation(out=sd4[:, :nwin],
                                 in_=mv4[:, :nwin, 1:2], func=Act.Sqrt,
                                 bias=eps_sb[:, 0:1], scale=1.0)
            rs4 = tailp.tile([P, WB], f32, tag="rs4")
            nc.vector.reciprocal(out=rs4[:, :nwin], in_=sd4[:, :nwin])
            # nmr = -mean * rs  (per window column)
            nmr = tailp.tile([P, WB], f32, tag="nmr")
            nc.vector.scalar_tensor_tensor(
                out=nmr[:, :nwin], in0=mv4[:, :nwin, 0], scalar=-1.0,
                in1=rs4[:, :nwin], op0=Alu.mult, op1=Alu.mult)
            # v = u * rs + nmr on the scalar engine (per-partition scale+bias)
            v4 = tailp.tile([P, WB, D], f32, tag="v4")
            for wi in range(nwin):
                nc.scalar.activation(
                    out=v4[:, wi, :], in_=u[:, wi, :], func=Act.Identity,
                    bias=nmr[:, wi:wi + 1], scale=rs4[:, wi:wi + 1])
            vout = v4
            if not cfg["ln_trivial"]:
                v2t = tailp.tile([P, WB, D], f32, tag="v2t")
                for wi in range(nwin):
                    nc.vector.tensor_tensor(out=v2t[:, wi, :],
                                            in0=v4[:, wi, :], in1=lng_bc,
                                            op=Alu.mult)
                    nc.vector.tensor_tensor(out=v2t[:, wi, :],
                                            in0=v2t[:, wi, :], in1=lnb_bc,
                                            op=Alu.add)
                vout = v2t
            nc.sync.dma_start(
                out=bass.AP(tensor=y_d, offset=w0 * P * D,
                            ap=[[D, P], [P * D, nwin], [1, D]]),
                in_=vout[:, :nwin, :])

        # gathers: one dma_gather per (block, quarter) run, split into
        # <=GCH-tile chunks (larger single gathers overflow the SWDGE ring).
        # Emitted just-in-time from the batch loop, AFTER the phase-1 writes
        # of their quarter (deps are tracked in program order).
        GCH = 8
        xs_run = {}
        gq = 0
        next_run = [0]

        def emit_runs_until(tile_limit):
            nonlocal gq
            while next_run[0] < len(runs):
                ri = next_run[0]
                q, r0, ntl = runs[ri]
                if r0 >= tile_limit:
                    return
                while ph1_done[0] < (q + 1) * qbat:
                    emit_ph1(1)
                gx = xrun.tile([P, max_run, DS], f32, tag="gx",
                               name=f"gx{ri}")
                for o in range(0, ntl, GCH):
                    n = min(GCH, ntl - o)
                    nc.gpsimd.dma_gather(
                        out_ap=gx[:, o:o + n, :],
                        in_ap=xsdq[q][0:SHARD, :],
                        idxs_ap=gidxw_sb[:, (r0 + o) * TB:
                                         (r0 + o + n) * TB],
                        num_idxs=n * P, num_idxs_reg=n * P, elem_size=DS,
                        queue_num=gq % 4)
                    gq += 1
                xs_run[ri] = (gx, r0)
                next_run[0] += 1

        psc = {}       # block -> psum tile [P, WB*P]
        for bph in range(T // TB):
            emit_ph1(4)
            emit_runs_until((bph + 4) * TB)
            t0 = bph * TB
            cat = catp.tile([P, TB, P], bf, tag="cat")
            # rbf + f1 + silu for 2 groups of 4 tiles
            hs = []
            for g in range(2):
                gg = bph * 2 + g
                rbf_sb = rbfsp.tile([P, P], bf, tag="rbf")
                nc.sync.dma_start(out=rbf_sb,
                                  in_=rbfp[:, gg * P:(gg + 1) * P])
                ph = hps.tile([P, 4 * P], f32, tag="h")
                for a in range(4):
                    nc.tensor.matmul(
                        out=ph[:, a * P:(a + 1) * P],
                        lhsT=f1wTp_sb[:, a * P:(a + 1) * P],
                        rhs=rbf_sb, start=True, stop=True)
                h_sb = hp.tile([P, 4 * P], bf, tag="h_sb")
                nc.scalar.activation(out=h_sb, in_=ph, func=Act.Silu,
                                     bias=f1b_sb[:, 0:1], scale=1.0)
                hs.append(h_sb)
            pphi = phips.tile([P, TB * DS], f32, tag="phi")
            for j in range(TB):
                nc.tensor.matmul(out=pphi[:, j * DS:(j + 1) * DS],
                                 lhsT=hs[j // 4][:, (j % 4) * P:(j % 4 + 1) * P],
                                 rhs=f2wT_sb, start=True, stop=True)
            phi3 = pphi.rearrange("p (j s) -> p j s", s=DS)
            # per-run segments: cast xs into cat cols 64:128, then multiply
            # by phi read directly from PSUM in f32 (better precision)
            for (j_lo, j_hi, ri) in segments[bph]:
                gx, r0 = xs_run[ri]
                o = t0 + j_lo - r0
                nc.scalar.activation(
                    out=cat[:, j_lo:j_hi, DS:P],
                    in_=gx[:, o:o + (j_hi - j_lo), :], func=Act.Copy)
                nc.vector.tensor_tensor(
                    out=cat[:, j_lo:j_hi, 0:DS],
                    in0=phi3[:, j_lo:j_hi, :],
                    in1=cat[:, j_lo:j_hi, DS:P],
                    op=Alu.mult)
            S8a = sp.tile([P, TB, P], bf, tag="S8a")
            nc.vector.tensor_tensor(
                out=S8a,
                in0=iota8_sb.rearrange("p (a b) -> p a b", b=P),
                in1=tgtl_sb[:, t0:t0 + TB].unsqueeze(2)
                .broadcast_to([P, TB, P]),
                op=Alu.is_equal)
            S8 = sp.tile([P, TB, P], bf, tag="S8")
            nc.vector.tensor_tensor(
                out=S8, in0=S8a,
                in1=envs_sb[:, t0:t0 + TB].unsqueeze(2)
                .broadcast_to([P, TB, P]),
                op=Alu.mult)
            for j in range(TB):
                t = t0 + j
                w = tile_win[t]
                blk = w // WB
                if blk not in psc:
                    psc[blk] = scps.tile([P, WB * P], f32, tag="sc",
                                         name=f"psc{blk}")
                wi = w % WB
                nc.tensor.matmul(out=psc[blk][:, wi * P:(wi + 1) * P],
                                 lhsT=cat[:, j, :], rhs=S8[:, j, :],
                                 start=(t == win_first[w]),
                                 stop=(t == win_last[w]),
                                 skip_group_check=True)
                if t == blk_last[blk]:
                    tail_block(blk, psc[blk])
                    psc.pop(blk, None)

    nc.compile()
    return nc


# --------------------------------------------------------------------------
# Entry point
# --------------------------------------------------------------------------

def _run(inputs, trace=False, n_cores=8, tmpdir=None):
    import time as _time
    t0 = _time.time()
    cfg, shared, per_core = _preprocess(n_cores=n_cores, **inputs)
    t1 = _time.time()
    nc = _build_program(cfg)
    t2 = _time.time()

    from concourse.bass_utils import run_bass_kernel_spmd

    in_maps = []
    for c in range(n_cores):
        m = dict(shared)
        m.update(per_core[c])
        in_maps.append(m)
    res = run_bass_kernel_spmd(nc, in_maps, core_ids=list(range(n_cores)),
                               trace=trace, tmpdir=tmpdir)
    t3 = _time.time()
    print(f"[kernel] preprocess {t1 - t0:.1f}s  build {t2 - t1:.1f}s  "
          f"compile+run {t3 - t2:.1f}s", file=sys.stderr)
    out = np.concatenate([res.results[c]["y"][:cfg["NS"]]
                          for c in range(n_cores)], axis=0)
    return out[:cfg["N"]].astype(np.float32), res


def kernel(**inputs):
    return _run(inputs)[0]



# revision 3
# speedup vs baseline: 1.0769x; 1.0769x over previous
"""Trainium2 Bass kernel for nn_CopresheafLayer (GNN message passing).

Math (reference):
    h     = silu(rbf @ f1_w.T + f1_b)                    # [E, 128]
    phi   = (h @ f2_w.T + f2_b) * envelope[:, None]      # [E, 64]
    msg   = (phi * (x[src] @ W_send.T)) @ W_recv         # [E, 128]
    agg   = segment_sum(msg, tgt, N)                     # [N, 128]
    gate  = silu(agg @ g1_w.T + g1_b) @ g2_w.T + g2_b
    y     = LayerNorm(x + gate) * ln_g + ln_b

Key algebra used by the kernel:
  * (sum_e stalk_msg_e) @ W_recv  ==  (sum_e stalk_msg_e) @ W_recv: the
    [E,64]@[64,128] matmul is hoisted to per-node; edges only touch the
    64-dim stalk space.
  * x[src] @ W_send.T == (x @ W_send.T)[src]: project once to xs=[N,64] f32,
    gather 256B rows per edge with dma_gather.
  * envelope is folded into the one-hot scatter matrix S (S = onehot * env).
  * f2_b: scatter BOTH phi*xs and raw xs (one [128,128] stationary operand),
    fold f2_b into an augmented W_recv in the per-node tail:
        agg @ W_recv = [scat_msg ; scat_xs] @ [W_recv ; f2_b[:,None]*W_recv]

Sharding: edges bucketed by target-node range (N/8 nodes per core), sorted by
tgt -> each core owns its output slice; no collectives.  segment_sum is a
sequence of one-hot matmuls accumulating into per-128-node-window PSUM
columns.  Edges are grouped (window, src-quarter)-pure and 128-padded so each
(4-window-block, quarter) is one contiguous dma_gather from an int16-indexed
shard of the xs table.
"""

import math
import os
import sys

import numpy as np

sys.path.insert(0, "/opt/trn_rl_repo")

P = 128  # SBUF partitions
D = 128  # d_node
DS = 64  # d_stalk
R = 32  # n_rbf
TB = 8  # tiles per phi batch (1024 edges)
WB = 4  # windows per scatter block
NQ = 4  # xs-table shards (int16 index range)


# --------------------------------------------------------------------------
# Host-side preprocessing (index manipulation only -- no model FLOPs)
# --------------------------------------------------------------------------

def _preprocess(x, edge_index, rbf, envelope, W_send, W_recv, f1_w, f1_b,
                f2_w, f2_b, g1_w, g1_b, g2_w, g2_b, ln_g, ln_b, n_cores):
    import ml_dtypes

    bf16 = ml_dtypes.bfloat16
    N = x.shape[0]
    E = edge_index.shape[1]
    assert N % n_cores == 0
    NS = N // n_cores                 # nodes per core
    NW = (NS + P - 1) // P            # 128-node windows per core
    NCHUNK = -(-N // P)
    # multiple of 4*32 so each xs-table quarter is whole 32-chunk batches
    NCHUNK = -(-NCHUNK // 128) * 128
    SHARD = (P * NCHUNK) // NQ        # xs-table rows per int16 shard
    assert SHARD <= 32768

    src = np.asarray(edge_index[0]).astype(np.int64)
    tgt = np.asarray(edge_index[1]).astype(np.int64)
    env = np.asarray(envelope, dtype=np.float32)
    rbf = np.asarray(rbf, dtype=np.float32)

    order = np.argsort(tgt, kind="stable")
    tgt_s, src_s, env_s = tgt[order], src[order], env[order]
    q_s = src_s // SHARD                          # shard per edge (node order)
    core_of = tgt_s // NS
    win_s = (tgt_s % NS) // P

    # per (core, window, quarter) counts -> shared group structure
    cnt = np.zeros((n_cores, NW, NQ), dtype=np.int64)
    np.add.at(cnt, (core_of, win_s, q_s), 1)
    maxcnt = cnt.max(axis=0)                      # real slots to gather
    G = -(-maxcnt // P)                           # tiles per (window, quarter)

    # block/quarter-major tile layout
    nblk = -(-NW // WB)
    grp_start = np.zeros((NW, NQ), dtype=np.int64)
    runs = []          # (q, tile_start, ntiles) per (block, quarter)
    run_groups = []    # per run: [(tile_start_abs, num_idxs), ...]
    win_first = np.full(NW, -1, dtype=np.int64)
    win_last = np.zeros(NW, dtype=np.int64)
    tile_win = []
    t = 0
    for b in range(nblk):
        ws = range(b * WB, min((b + 1) * WB, NW))
        for q in range(NQ):
            r0 = t
            groups = []
            for w in ws:
                g = int(G[w, q])
                if g == 0:
                    continue
                grp_start[w, q] = t
                groups.append((t, int(maxcnt[w, q])))
                if win_first[w] < 0:
                    win_first[w] = t
                win_last[w] = t + g - 1
                tile_win += [w] * g
                t += g
            if b == nblk - 1 and q == NQ - 1:
                pad = (-t) % TB                   # T multiple of TB
                if pad:
                    w = NW - 1
                    tile_win += [w] * pad
                    win_last[w] = t + pad - 1
                    t += pad
            if t > r0:
                runs.append((q, r0, t - r0))
                run_groups.append(groups)
    T = t
    assert all(f >= 0 for f in win_first)

    # msg-op segments: phi batches split at run boundaries
    run_of_tile = np.zeros(T, dtype=np.int64)
    run_t0 = np.zeros(len(runs), dtype=np.int64)
    for ri, (q, r0, n) in enumerate(runs):
        run_of_tile[r0:r0 + n] = ri
        run_t0[ri] = r0
    segments = []      # per batch: list of (j_lo, j_hi, run_idx)
    for bph in range(T // TB):
        segs, t0 = [], bph * TB
        j = 0
        while j < TB:
            ri = run_of_tile[t0 + j]
            j2 = j
            while j2 < TB and run_of_tile[t0 + j2] == ri:
                j2 += 1
            segs.append((j, j2, int(ri)))
            j = j2
        segments.append(segs)

    per_core = []
    for c in range(n_cores):
        slots = T * P
        src_slot = np.zeros(slots, dtype=np.int64)
        env_slot = np.zeros(slots, dtype=np.float32)
        tgtl_slot = np.zeros(slots, dtype=np.float32)
        sel = core_of == c
        tw, tq = win_s[sel], q_s[sel]
        # position within the (w, q) group (vectorized cumcount)
        gkey = tw * NQ + tq
        pos = np.zeros(len(gkey), dtype=np.int64)
        so = np.argsort(gkey, kind="stable")
        gs = gkey[so]
        pos[so] = np.arange(len(gs)) - np.searchsorted(gs, gs)
        slot = grp_start[tw, tq] * P + pos
        src_slot[slot] = src_s[sel]
        env_slot[slot] = env_s[sel]
        tgtl_slot[slot] = (tgt_s[sel] - c * NS - tw * P).astype(np.float32)
        rbf_slot = np.zeros((slots, R), dtype=np.float32)
        rbf_slot[slot] = rbf[order[sel]]

        rel = np.zeros(slots, dtype=np.int16)
        rel[slot] = (src_s[sel] % SHARD).astype(np.int16)
        # wrapped idx layout: idx i -> partition i%16, col i//16, replicated
        idxw = rel.reshape(T * TB, 16).T            # [16, T*8]
        idxw = np.tile(idxw, (8, 1))                # [128, T*8]
        maps = {
            "gidxw": np.ascontiguousarray(idxw),
            "tgtlenv": np.ascontiguousarray(np.concatenate(
                [tgtl_slot.reshape(T, P).T, env_slot.reshape(T, P).T],
                axis=1).astype(bf16)),
            # packed rbf^T: [32a+k, g*128+j] = rbf_slot[g*512+a*128+j, k]
            "rbfp": np.ascontiguousarray(
                rbf_slot.reshape(T // 4, 4, P, R).transpose(1, 3, 0, 2)
                .reshape(P, (T // 4) * P).astype(bf16)),
            "xres": np.ascontiguousarray(np.concatenate([
                np.asarray(x[c * NS:(c + 1) * NS], np.float32)
                + np.asarray(g2_b, np.float32)[None, :],
                np.zeros((NW * P - NS, x.shape[1]), np.float32)], axis=0)),
        }
        per_core.append(maps)

    xT = np.zeros((D, NCHUNK * P), dtype=bf16)
    xT[:, :N] = np.asarray(x, np.float32).T.astype(bf16)
    f2b = np.asarray(f2_b, np.float32)
    wrecv = np.asarray(W_recv, np.float32)
    f1wT = np.asarray(f1_w, np.float32).T           # [32, 128]
    f1wTp = np.zeros((P, 4 * P), dtype=np.float32)  # zero-padded copies
    for a in range(4):
        f1wTp[a * R:(a + 1) * R, a * P:(a + 1) * P] = f1wT
    shared = {
        "xT": xT,
        "f1wTp": f1wTp.astype(bf16),
        "f2wT": np.ascontiguousarray(np.asarray(f2_w, np.float32).T.astype(bf16)),
        "wrecvaug": np.ascontiguousarray(
            np.vstack([wrecv, f2b[:, None] * wrecv]).astype(bf16)),
        "g1wT": np.ascontiguousarray(np.asarray(g1_w, np.float32).T.astype(bf16)),
        "g2wT": np.ascontiguousarray(np.asarray(g2_w, np.float32).T.astype(bf16)),
        "wsendT": np.ascontiguousarray(np.asarray(W_send, np.float32).T.astype(bf16)),
        "f1b": np.asarray(f1_b, np.float32).reshape(D, 1).copy(),
        "g1b": np.asarray(g1_b, np.float32).reshape(D, 1).copy(),
        "iota": np.ascontiguousarray(
            np.broadcast_to(np.arange(P, dtype=np.float32)[None, :], (P, P))
            .astype(bf16)),
        "iota8": np.ascontiguousarray(
            np.broadcast_to(
                np.arange(P, dtype=np.float32)[None, None, :],
                (P, TB, P)).reshape(P, TB * P).astype(bf16)),
    }
    ln_g = np.asarray(ln_g, np.float32)
    ln_b = np.asarray(ln_b, np.float32)
    ln_trivial = bool(np.all(ln_g == 1.0) and np.all(ln_b == 0.0))
    if not ln_trivial:
        shared["lngrow"] = ln_g.reshape(1, D).copy()
        shared["lnbrow"] = ln_b.reshape(1, D).copy()

    cfg = dict(N=N, E=E, NS=NS, NW=NW, NCHUNK=NCHUNK, T=T, SHARD=SHARD,
               runs=runs, run_groups=run_groups, segments=segments,
               win_first=win_first.tolist(), win_last=win_last.tolist(),
               tile_win=[int(v) for v in tile_win], ln_trivial=ln_trivial,
               n_cores=n_cores)
    return cfg, shared, per_core


# --------------------------------------------------------------------------
# Bass program (identical across cores)
# --------------------------------------------------------------------------

def _build_program(cfg):
    from contextlib import ExitStack

    import concourse.bacc as bacc
    import concourse.bass as bass
    import concourse.tile as tile
    from concourse import mybir

    f32 = mybir.dt.float32
    bf = mybir.dt.bfloat16
    i16 = mybir.dt.int16
    Alu = mybir.AluOpType
    Act = mybir.ActivationFunctionType

    NS, NW, NCH, T = cfg["NS"], cfg["NW"], cfg["NCHUNK"], cfg["T"]
    SHARD = cfg["SHARD"]
    runs = cfg["runs"]
    run_groups = cfg["run_groups"]
    segments = cfg["segments"]
    win_first, win_last = cfg["win_first"], cfg["win_last"]
    tile_win = cfg["tile_win"]
    last_nw = NS - P * (NW - 1)

    nc = bacc.Bacc("TRN2", target_bir_lowering=False, debug=False,
                   num_swdge_queues=4)

    rbfp = nc.dram_tensor("rbfp", [P, (T // 4) * P], bf, kind="ExternalInput")
    gidxw_d = nc.dram_tensor("gidxw", [P, T * TB], i16, kind="ExternalInput")
    tgtlenv_d = nc.dram_tensor("tgtlenv", [P, 2 * T], bf, kind="ExternalInput")
    iota8_d = nc.dram_tensor("iota8", [P, TB * P], bf, kind="ExternalInput")
    xT_d = nc.dram_tensor("xT", [D, NCH * P], bf, kind="ExternalInput")
    xres_d = nc.dram_tensor("xres", [NW * P, D], f32, kind="ExternalInput")
    f1wTp_d = nc.dram_tensor("f1wTp", [P, 4 * P], bf, kind="ExternalInput")
    f2wT_d = nc.dram_tensor("f2wT", [D, DS], bf, kind="ExternalInput")
    wrecvaug_d = nc.dram_tensor("wrecvaug", [P, D], bf, kind="ExternalInput")
    g1wT_d = nc.dram_tensor("g1wT", [D, D], bf, kind="ExternalInput")
    g2wT_d = nc.dram_tensor("g2wT", [D, D], bf, kind="ExternalInput")
    wsendT_d = nc.dram_tensor("wsendT", [D, DS], bf, kind="ExternalInput")
    f1b_d = nc.dram_tensor("f1b", [D, 1], f32, kind="ExternalInput")
    g1b_d = nc.dram_tensor("g1b", [D, 1], f32, kind="ExternalInput")
    iota_d = nc.dram_tensor("iota", [P, P], bf, kind="ExternalInput")
    if not cfg["ln_trivial"]:
        lngrow_d = nc.dram_tensor("lngrow", [1, D], f32, kind="ExternalInput")
        lnbrow_d = nc.dram_tensor("lnbrow", [1, D], f32, kind="ExternalInput")
    y_d = nc.dram_tensor("y", [NW * P, D], f32, kind="ExternalOutput")

    with ExitStack() as ctx:
        tc = ctx.enter_context(tile.TileContext(nc))
        consts = ctx.enter_context(tc.tile_pool(name="consts", bufs=1))
        dram = ctx.enter_context(tc.tile_pool(name="dram", bufs=1, space="DRAM"))

        def load_const(dt_tensor, shape, dtype, name):
            t = consts.tile(shape, dtype, name=name)
            nc.sync.dma_start(out=t, in_=dt_tensor[:, :])
            return t

        f1wTp_sb = load_const(f1wTp_d, [P, 4 * P], bf, "f1wTp_sb")
        f2wT_sb = load_const(f2wT_d, [D, DS], bf, "f2wT_sb")
        wrecvaug_sb = load_const(wrecvaug_d, [P, D], bf, "wrecvaug_sb")
        g1wT_sb = load_const(g1wT_d, [D, D], bf, "g1wT_sb")
        g2wT_sb = load_const(g2wT_d, [D, D], bf, "g2wT_sb")
        wsendT_sb = load_const(wsendT_d, [D, DS], bf, "wsendT_sb")
        f1b_sb = load_const(f1b_d, [D, 1], f32, "f1b_sb")
        g1b_sb = load_const(g1b_d, [D, 1], f32, "g1b_sb")
        iota_sb = load_const(iota_d, [P, P], bf, "iota_sb")
        eps_sb = consts.tile([P, 1], f32, name="eps_sb")
        nc.vector.memset(eps_sb, 1e-5)
        gidxw_sb = load_const(gidxw_d, [P, T * TB], i16, "gidxw_sb")
        tgtlenv_sb = load_const(tgtlenv_d, [P, 2 * T], bf, "tgtlenv_sb")
        tgtl_sb = tgtlenv_sb[:, 0:T]
        envs_sb = tgtlenv_sb[:, T:2 * T]
        iota8_sb = load_const(iota8_d, [P, TB * P], bf, "iota8_sb")
        # tiny DVE touch of iota so the first S-build op doesn't carry its
        # DMA wait (TensorScalar ISA allows only 2 sync waits)
        iota_touch = consts.tile([P, 1], f32, name="iota_touch")
        nc.vector.tensor_copy(out=iota_touch, in_=iota_sb[:, 0:1])
        if not cfg["ln_trivial"]:
            lng_bc = consts.tile([P, D], f32, name="lng_bc")
            nc.sync.dma_start(
                out=lng_bc,
                in_=bass.AP(tensor=lngrow_d, offset=0, ap=[[0, P], [1, D]]))
            lnb_bc = consts.tile([P, D], f32, name="lnb_bc")
            nc.sync.dma_start(
                out=lnb_bc,
                in_=bass.AP(tensor=lnbrow_d, offset=0, ap=[[0, P], [1, D]]))

        # xs table in DRAM (f32), node-order rows, one tile per quarter so
        # gathers for quarter q only depend on 1/4 of phase 1
        xsdq = [dram.tile([SHARD, DS], f32, name=f"xsdq{q}")
                for q in range(NQ)]

        # ---------------- phase 1: xs = x @ W_send.T (f32) ----------------
        # Emitted lazily: quarters 0-1 up front, the rest interleaved into
        # the phase-2 batch loop so engine queues aren't clogged by phase 1.
        PH1TB = 32
        qbat = (NCH // PH1TB) // NQ    # 16-chunk batches per quarter
        ph1 = ctx.enter_context(tc.tile_pool(name="ph1", bufs=3))

        tps = ctx.enter_context(tc.tile_pool(name="tps", bufs=2,
                                              space="PSUM"))

        def emit_ph1_batch(cb):
            q = cb // qbat
            xt_sb = ph1.tile([D, PH1TB * P], bf, tag="xt")
            nc.sync.dma_start(
                out=xt_sb,
                in_=xT_d[:, cb * PH1TB * P:(cb + 1) * PH1TB * P])
            xs_sb = ph1.tile([P, PH1TB * DS], f32, tag="xs")
            for h in range(PH1TB // TB):
                ps = tps.tile([P, WB * P], f32, tag="tp")
                for j in range(TB):
                    nc.tensor.matmul(
                        out=ps[:, j * DS:(j + 1) * DS],
                        lhsT=xt_sb[:, (h * TB + j) * P:
                                   (h * TB + j + 1) * P],
                        rhs=wsendT_sb, start=True, stop=True)
                nc.scalar.activation(
                    out=xs_sb[:, h * TB * DS:(h + 1) * TB * DS],
                    in_=ps, func=Act.Copy)
            # write to node-order rows of quarter q: (p, j, s) ->
            # local row (cb*PH1TB+j)*128 + p - q*SHARD  (scalar HWDGE queue
            # so the ramp's DMA issue isn't serialized on sync)
            nc.scalar.dma_start(
                out=bass.AP(tensor=xsdq[q].tensor,
                            offset=(cb - q * qbat) * PH1TB * P * DS,
                            ap=[[DS, P], [P * DS, PH1TB], [1, DS]]),
                in_=xs_sb.rearrange("p (j s) -> p j s", s=DS))

        ph1_iter = iter(range(NCH // PH1TB))
        ph1_done = [0]

        def emit_ph1(n):
            for _ in range(n):
                cb = next(ph1_iter, None)
                if cb is None:
                    return
                emit_ph1_batch(cb)
                ph1_done[0] += 1

        emit_ph1(NCH // PH1TB)         # all of phase 1 up front

        # ---------------- phase 2: edges ----------------
        catp = ctx.enter_context(tc.tile_pool(name="catp", bufs=3))
        xrun = ctx.enter_context(tc.tile_pool(name="xrun", bufs=6))
        rbfsp = ctx.enter_context(tc.tile_pool(name="rbfsp", bufs=4))
        hp = ctx.enter_context(tc.tile_pool(name="hp", bufs=3))
        phip = ctx.enter_context(tc.tile_pool(name="phip", bufs=3))
        sp = ctx.enter_context(tc.tile_pool(name="sp", bufs=4))
        tailp = ctx.enter_context(tc.tile_pool(name="tailp", bufs=2))
        hps = ctx.enter_context(tc.tile_pool(name="hps", bufs=2, space="PSUM"))
        phips = ctx.enter_context(tc.tile_pool(name="phips", bufs=2, space="PSUM"))
        scps = ctx.enter_context(tc.tile_pool(name="scps", bufs=2, space="PSUM"))

        max_run = max(n for (_, _, n) in runs)
        nblk = -(-NW // WB)
        blk_last = [max(win_last[w]
                        for w in range(b * WB, min((b + 1) * WB, NW)))
                    for b in range(nblk)]

        def tail_block(blk, psc_tile):
            w0 = blk * WB
            nwin = min(WB, NW - w0)
            W = nwin * P
            sc_sb = tailp.tile([P, WB * P], bf, tag="sc")
            nc.scalar.activation(out=sc_sb[:, :W], in_=psc_tile[:, :W],
                                 func=Act.Copy)
            pB = tps.tile([P, WB * P], f32, tag="tp")
            nc.tensor.matmul(out=pB[:, :W], lhsT=wrecvaug_sb,
                             rhs=sc_sb[:, :W], start=True, stop=True)
            B_sb = tailp.tile([P, WB * P], bf, tag="B")
            nc.scalar.activation(out=B_sb[:, :W], in_=pB[:, :W], func=Act.Copy)
            pC = tps.tile([P, WB * P], f32, tag="tp")
            nc.tensor.matmul(out=pC[:, :W], lhsT=g1wT_sb, rhs=B_sb[:, :W],
                             start=True, stop=True)
            C_sb = tailp.tile([P, WB * P], bf, tag="C")
            nc.scalar.activation(out=C_sb[:, :W], in_=pC[:, :W], func=Act.Silu,
                                 bias=g1b_sb[:, 0:1], scale=1.0)
            pG = tps.tile([P, WB * P], f32, tag="tp")
            for wi in range(nwin):
                nc.tensor.matmul(out=pG[:, wi * P:(wi + 1) * P],
                                 lhsT=C_sb[:, wi * P:(wi + 1) * P],
                                 rhs=g2wT_sb, start=True, stop=True)
            xw = tailp.tile([P, WB, D], f32, tag="xw")
            nc.sync.dma_start(
                out=xw[:, :nwin, :],
                in_=bass.AP(tensor=xres_d, offset=w0 * P * D,
                            ap=[[D, P], [P * D, nwin], [1, D]]))
            u = tailp.tile([P, WB, D], f32, tag="u")
            nc.vector.scalar_tensor_tensor(
                out=u[:, :nwin, :],
                in0=pG.rearrange("p (a b) -> p a b", b=D)[:, :nwin, :],
                scalar=1.0, in1=xw[:, :nwin, :], op0=Alu.mult, op1=Alu.add)
            st4 = tailp.tile([P, WB, 6], f32, tag="st4")
            mv4 = tailp.tile([P, WB, 2], f32, tag="mv4")
            for wi in range(nwin):
                nc.vector.bn_stats(out=st4[:, wi, :], in_=u[:, wi, :])
                nc.vector.bn_aggr(out=mv4[:, wi, :], in_=st4[:, wi, :])
            sd4 = tailp.tile([P, WB], f32, tag="sd4")
            nc.scalar.activation(out=sd4[:, :nwin],
                                 in_=mv4[:, :nwin, 1:2], func=Act.Sqrt,
                                 bias=eps_sb[:, 0:1], scale=1.0)
            rs4 = tailp.tile([P, WB], f32, tag="rs4")
            nc.vector.reciprocal(out=rs4[:, :nwin], in_=sd4[:, :nwin])
            # nmr = -mean * rs  (per window column)
            nmr = tailp.tile([P, WB], f32, tag="nmr")
            nc.vector.scalar_tensor_tensor(
                out=nmr[:, :nwin], in0=mv4[:, :nwin, 0], scalar=-1.0,
                in1=rs4[:, :nwin], op0=Alu.mult, op1=Alu.mult)
            # v = u * rs + nmr on the scalar engine (per-partition scale+bias)
            v4 = tailp.tile([P, WB, D], f32, tag="v4")
            for wi in range(nwin):
                nc.scalar.activation(
                    out=v4[:, wi, :], in_=u[:, wi, :], func=Act.Identity,
                    bias=nmr[:, wi:wi + 1], scale=rs4[:, wi:wi + 1])
            vout = v4
            if not cfg["ln_trivial"]:
                v2t = tailp.tile([P, WB, D], f32, tag="v2t")
                for wi in range(nwin):
                    nc.vector.tensor_tensor(out=v2t[:, wi, :],
                                            in0=v4[:, wi, :], in1=lng_bc,
                                            op=Alu.mult)
                    nc.vector.tensor_tensor(out=v2t[:, wi, :],
                                            in0=v2t[:, wi, :], in1=lnb_bc,
                                            op=Alu.add)
                vout = v2t
            nc.sync.dma_start(
                out=bass.AP(tensor=y_d, offset=w0 * P * D,
                            ap=[[D, P], [P * D, nwin], [1, D]]),
                in_=vout[:, :nwin, :])

        # gathers: one dma_gather per (block, quarter) run, split into
        # <=GCH-tile chunks (larger single gathers overflow the SWDGE ring).
        # Emitted just-in-time from the batch loop, AFTER the phase-1 writes
        # of their quarter (deps are tracked in program order).
        GCH = 8
        xs_run = {}
        gq = 0
        next_run = [0]

        def emit_runs_until(tile_limit):
            nonlocal gq
            while next_run[0] < len(runs):
                ri = next_run[0]
                q, r0, ntl = runs[ri]
                if r0 >= tile_limit:
                    return
                while ph1_done[0] < (q + 1) * qbat:
                    emit_ph1(1)
                gx = xrun.tile([P, max_run, DS], f32, tag="gx",
                               name=f"gx{ri}")
                for o in range(0, ntl, GCH):
                    n = min(GCH, ntl - o)
                    nc.gpsimd.dma_gather(
                        out_ap=gx[:, o:o + n, :],
                        in_ap=xsdq[q][0:SHARD, :],
                        idxs_ap=gidxw_sb[:, (r0 + o) * TB:
                                         (r0 + o + n) * TB],
                        num_idxs=n * P, num_idxs_reg=n * P, elem_size=DS,
                        queue_num=gq % 4)
                    gq += 1
                xs_run[ri] = (gx, r0)
                next_run[0] += 1

        psc = {}       # block -> psum tile [P, WB*P]
        for bph in range(T // TB):
            emit_ph1(4)
            emit_runs_until((bph + 9) * TB)
            t0 = bph * TB
            cat = catp.tile([P, TB, P], bf, tag="cat")
            # rbf + f1 + silu for 2 groups of 4 tiles
            hs = []
            for g in range(2):
                gg = bph * 2 + g
                rbf_sb = rbfsp.tile([P, P], bf, tag="rbf")
                nc.sync.dma_start(out=rbf_sb,
                                  in_=rbfp[:, gg * P:(gg + 1) * P])
                ph = hps.tile([P, 4 * P], f32, tag="h")
                for a in range(4):
                    nc.tensor.matmul(
                        out=ph[:, a * P:(a + 1) * P],
                        lhsT=f1wTp_sb[:, a * P:(a + 1) * P],
                        rhs=rbf_sb, start=True, stop=True)
                h_sb = hp.tile([P, 4 * P], bf, tag="h_sb")
                nc.scalar.activation(out=h_sb, in_=ph, func=Act.Silu,
                                     bias=f1b_sb[:, 0:1], scale=1.0)
                hs.append(h_sb)
            pphi = phips.tile([P, TB * DS], f32, tag="phi")
            for j in range(TB):
                nc.tensor.matmul(out=pphi[:, j * DS:(j + 1) * DS],
                                 lhsT=hs[j // 4][:, (j % 4) * P:(j % 4 + 1) * P],
                                 rhs=f2wT_sb, start=True, stop=True)
            phi3 = pphi.rearrange("p (j s) -> p j s", s=DS)
            # per-run segments: cast xs into cat cols 64:128, then multiply
            # by phi read directly from PSUM in f32 (better precision)
            for (j_lo, j_hi, ri) in segments[bph]:
                gx, r0 = xs_run[ri]
                o = t0 + j_lo - r0
                nc.scalar.activation(
                    out=cat[:, j_lo:j_hi, DS:P],
                    in_=gx[:, o:o + (j_hi - j_lo), :], func=Act.Copy)
                nc.vector.tensor_tensor(
                    out=cat[:, j_lo:j_hi, 0:DS],
                    in0=phi3[:, j_lo:j_hi, :],
                    in1=cat[:, j_lo:j_hi, DS:P],
                    op=Alu.mult)
            S8a = sp.tile([P, TB, P], bf, tag="S8a")
            nc.vector.tensor_tensor(
                out=S8a,
                in0=iota8_sb.rearrange("p (a b) -> p a b", b=P),
                in1=tgtl_sb[:, t0:t0 + TB].unsqueeze(2)
                .broadcast_to([P, TB, P]),
                op=Alu.is_equal)
            S8 = sp.tile([P, TB, P], bf, tag="S8")
            nc.vector.tensor_tensor(
                out=S8, in0=S8a,
                in1=envs_sb[:, t0:t0 + TB].unsqueeze(2)
                .broadcast_to([P, TB, P]),
                op=Alu.mult)
            for j in range(TB):
                t = t0 + j
                w = tile_win[t]
                blk = w // WB
                if blk not in psc:
                    psc[blk] = scps.tile([P, WB * P], f32, tag="sc",
                                         name=f"psc{blk}")
                wi = w % WB
                nc.tensor.matmul(out=psc[blk][:, wi * P:(wi + 1) * P],
                                 lhsT=cat[:, j, :], rhs=S8[:, j, :],
                                 start=(t == win_first[w]),
                                 stop=(t == win_last[w]),
                                 skip_group_check=True)
                if t == blk_last[blk]:
                    tail_block(blk, psc[blk])
                    psc.pop(blk, None)

    nc.compile()
    return nc


# --------------------------------------------------------------------------
# Entry point
# --------------------------------------------------------------------------

def _run(inputs, trace=False, n_cores=8, tmpdir=None):
    import time as _time
    t0 = _time.time()
    cfg, shared, per_core = _preprocess(n_cores=n_cores, **inputs)
    t1 = _time.time()
    nc = _build_program(cfg)
    t2 = _time.time()

    from concourse.bass_utils import run_bass_kernel_spmd

    in_maps = []
    for c in range(n_cores):
        m = dict(shared)
        m.update(per_core[c])
        in_maps.append(m)
    res = run_bass_kernel_spmd(nc, in_maps, core_ids=list(range(n_cores)),
                               trace=trace, tmpdir=tmpdir)
    t3 = _time.time()
    print(f"[kernel] preprocess {t1 - t0:.1f}s  build {t2 - t1:.1f}s  "
          f"compile+run {t3 - t2:.1f}s", file=sys.stderr)
    out = np.concatenate([res.results[c]["y"][:cfg["NS"]]
                          for c in range(n_cores)], axis=0)
    return out[:cfg["N"]].astype(np.float32), res


def kernel(**inputs):
    return _run(inputs)[0]

